# revision 2
# baseline (speedup 1.0000x reference)
"""Block-causal attention (B=4, N=2048, C=1024, H=16, block=128) on 8 TRN2 NeuronCores.

Sharding: core = 2*b + g  (b in 0..3 batches, g in 0..1 head-groups of 8 heads).
v5: all-bf16 matmul stream (v4) plus tile-reconfig-aware scheduling.
The PE pays ~107ns whenever consecutive matmuls change tile_size
(stationary KxM footprint).  v4 paid it 3x per attention step
(S 64x128 -> filler 128x128 -> AV/SM 128x64).  v5 processes key tiles
in PAIRS — S,S / exp,exp / filler burst / av,av,sm,sm (delayed two
steps) — halving the switch count and widening each same-shape run.
Weights/x load as 1-2 big strided DMAs each (fewer DMA-ring slots ->
no spurious startup waits); final out-DMAs go on the sync queue only
so the gpsimd end-of-block drain doesn't wait on them.
"""

import numpy as np
import ml_dtypes
from collections import deque
from contextlib import ExitStack

B, N, C, H, HD = 4, 2048, 1024, 16, 64
HPC = 8               # heads per core
F = HPC * HD          # 512 features per core
NCORES = 8
SCALE = float(HD) ** -0.5
NT = N // 128         # 16 token tiles
NCH = 4               # token chunks of 512

_CACHE = {}


def _build():
    import concourse.mybir as mybir
    import concourse.tile as tile
    from concourse import bacc

    f32 = mybir.dt.float32
    bf16 = mybir.dt.bfloat16
    Exp = mybir.ActivationFunctionType.Exp

    nc = bacc.Bacc("TRN2", target_bir_lowering=False, debug=False,
                   num_devices=NCORES)

    xT = nc.dram_tensor("xT", [C, N], bf16, kind="ExternalInput")
    wq = nc.dram_tensor("wq", [C, F], bf16, kind="ExternalInput")
    wk = nc.dram_tensor("wk", [C, F], bf16, kind="ExternalInput")
    wv = nc.dram_tensor("wv", [C, F], bf16, kind="ExternalInput")
    wp = nc.dram_tensor("wp", [F, C], bf16, kind="ExternalInput")
    ones_d = nc.dram_tensor("ones", [128, 64], bf16, kind="ExternalInput")
    out = nc.dram_tensor("out", [N, C], f32, kind="ExternalOutput")

    with tile.TileContext(nc) as tc, ExitStack() as ctx:
        persist = ctx.enter_context(tc.tile_pool(name="persist", bufs=1))
        xt_pool = ctx.enter_context(tc.tile_pool(name="xt", bufs=2))
        qt_pool = ctx.enter_context(tc.tile_pool(name="qt", bufs=2))
        at_pool = ctx.enter_context(tc.tile_pool(name="attnT", bufs=4))
        exp_pool = ctx.enter_context(tc.tile_pool(name="expT", bufs=4))
        rc_pool = ctx.enter_context(tc.tile_pool(name="recip", bufs=1))
        ost_pool = ctx.enter_context(tc.tile_pool(name="ost", bufs=2))
        ps_mm = ctx.enter_context(tc.tile_pool(name="ps_mm", bufs=2, space="PSUM"))
        ps_s = ctx.enter_context(tc.tile_pool(name="ps_s", bufs=2, space="PSUM"))
        ps_av = ctx.enter_context(tc.tile_pool(name="ps_av", bufs=1, space="PSUM"))
        ps_sum = ctx.enter_context(tc.tile_pool(name="ps_sum", bufs=1, space="PSUM"))

        # ---- persistent weights as single wide tiles (one DMA each) ----
        wq_t = persist.tile([128, 4096], bf16, name="wqall", tag="wqall")
        wk_t = persist.tile([128, 4096], bf16, name="wkall", tag="wkall")
        wv_t = persist.tile([128, 4096], bf16, name="wvall", tag="wvall")
        wp_t = persist.tile([128, 4096], bf16, name="wpall", tag="wpall")
        ones_t = persist.tile([128, 64], bf16, name="ones", tag="ones")

        def wsl(t, kk, lo, hi):
            return t[:, kk * 512 + lo:kk * 512 + hi]

        # persistent k^T (per head-pair per chunk) and v (per token tile)
        kt_t = [[persist.tile([128, 512], bf16, name=f"kT{hp}_{jc}", tag=f"kT{hp}_{jc}")
                 for jc in range(NCH)] for hp in range(4)]
        v_t = [persist.tile([128, F], bf16, name=f"v{t}", tag=f"v{t}") for t in range(NT)]

        def wload(q, dst, src, k0, k1, width=512):
            # one strided DMA: dram rows [k0*128,(k1)*128) -> sbuf cols
            dv = dst[:].rearrange("p (k f) -> p k f", f=width)[:, k0:k1, :]
            sv = src.rearrange("(k p) f -> p k f", p=128)[:, k0:k1, :]
            q.dma_start(dv, sv)

        # qt_state[c] / at_state[c] filled lazily by the emit units below
        qt_state = {c: [] for c in range(NCH)}
        at_state = {c: [] for c in range(NCH)}

        def qkv_units(c, split_xt=False):
            """Emission units for QKV of token chunk c: xt DMA + 12 groups,
            each flattened to 8 single-matmul units + 1 copy unit so they
            can be paced between attention steps at fine granularity."""
            c0 = c * 512
            xt_all = [None]

            def load():
                xt_all[0] = xt_pool.tile([128, 4096], bf16, name="xt",
                                         tag="xt")
                dv = xt_all[0][:].rearrange("p (k f) -> p k f", k=8)
                sv = xT[:, c0:c0 + 512].rearrange("(k p) f -> p k f", p=128)
                if split_xt:
                    nc.sync.dma_start(dv[:, 0:4], sv[:, 0:4])
                    nc.scalar.dma_start(dv[:, 4:8], sv[:, 4:8])
                else:
                    nc.sync.dma_start(dv, sv)

            def xsl(kk, lo=0, hi=512):
                return xt_all[0][:, kk * 512 + lo:kk * 512 + hi]

            units = [load]

            def group(mm, fin):
                cell = {}

                def mk(kk):
                    def emit():
                        if kk == 0:
                            cell["ps"] = ps_mm.tile([128, 512], f32,
                                                    name="mm", tag="mm")
                        mm(cell["ps"], kk)
                    return emit
                return [mk(kk) for kk in range(8)] + [lambda: fin(cell["ps"])]

            def k_group(hp):
                def mm(ps, kk):
                    nc.tensor.matmul(ps[:],
                                     wsl(wk_t, kk, hp * 128, (hp + 1) * 128),
                                     xsl(kk),
                                     start=(kk == 0), stop=(kk == 7))

                def fin(ps):
                    nc.vector.tensor_copy(kt_t[hp][c][:], ps[:])
                return group(mm, fin)

            def v_group(tl):
                def mm(ps, kk):
                    nc.tensor.matmul(ps[:],
                                     xsl(kk, tl * 128, (tl + 1) * 128),
                                     wsl(wv_t, kk, 0, 512),
                                     start=(kk == 0), stop=(kk == 7))

                def fin(ps):
                    nc.vector.tensor_copy(v_t[4 * c + tl][:], ps[:])
                return group(mm, fin)

            def q_group(hp):
                def mm(ps, kk):
                    nc.tensor.matmul(ps[:],
                                     wsl(wq_t, kk, hp * 128, (hp + 1) * 128),
                                     xsl(kk),
                                     start=(kk == 0), stop=(kk == 7))

                def fin(ps):
                    qt = qt_pool.tile([128, 512], bf16, name=f"qT{hp}",
                                      tag=f"qT{hp}")
                    nc.vector.tensor_copy(qt[:], ps[:])
                    qt_state[c].append(qt)
                return group(mm, fin)

            # k/v first (attention chunk c needs them for all j), q last
            for hp in range(4):
                units += k_group(hp)
            for tl in range(4):
                units += v_group(tl)
            for hp in range(4):
                units += q_group(hp)
            return units

        def proj_units(c, sync_only=False):
            """Partial projection of chunk c: 8 groups of 4 matmuls + fin."""
            units = []
            out_q = [nc.sync] if sync_only else [nc.sync, nc.gpsimd,
                                                 nc.scalar]

            def mk(tl, n2, kk, cell):
                def emit():
                    if kk == 0:
                        cell["ps"] = ps_mm.tile([128, 512], f32,
                                                name="mm", tag="mm")
                    nc.tensor.matmul(
                        cell["ps"][:],
                        at_state[c][kk][:, tl * 128:(tl + 1) * 128],
                        wp_t[:, kk * 1024 + n2 * 512:
                             kk * 1024 + (n2 + 1) * 512],
                        start=(kk == 0), stop=(kk == 3))
                return emit

            def fin(tl, n2, cell, qi):
                def emit():
                    t = 4 * c + tl
                    ost = ost_pool.tile([128, 512], f32, name="ost",
                                        tag="ost")
                    nc.vector.tensor_copy(ost[:], cell["ps"][:])
                    out_q[qi % len(out_q)].dma_start(
                        out[t * 128:(t + 1) * 128,
                            n2 * 512:(n2 + 1) * 512],
                        ost[:])
                return emit

            qi = 0
            for tl in range(4):
                for n2 in range(2):
                    cell = {}
                    for kk in range(4):
                        units.append(mk(tl, n2, kk, cell))
                    units.append(fin(tl, n2, cell, qi))
                    qi += 1
            return units

        # chunk 0's QKV has nothing to hide behind — emit it upfront, with
        # each weight DMA emitted right before the groups that use it.
        units0 = qkv_units(0, split_xt=True)
        units0[0]()           # xt chunk 0 (2 DMAs: sync + scalar)
        wload(nc.gpsimd, wk_t, wk, 0, 4)
        wload(nc.gpsimd, wk_t, wk, 4, 8)
        for u in units0[1:37]:          # 4 k-groups
            u()
        wload(nc.scalar, wv_t, wv, 0, 8)
        for u in units0[37:73]:         # 4 v-groups
            u()
        wload(nc.sync, wq_t, wq, 0, 8)
        for u in units0[73:]:           # 4 q-groups
            u()
        nc.gpsimd.dma_start(ones_t[:], ones_d[:])
        wload(nc.gpsimd, wp_t, wp, 0, 4, width=1024)

        def attn_unit(c, hp, fillers, stride):
            """Attention for (chunk c, head-pair hp), processed in key-tile
            PAIRS to minimize PE tile-size reconfigs: S,S / exp,exp /
            filler burst / av,av,sm,sm (delayed one pair = two tiles)."""
            njt = 4 * c + 4
            qt_c = qt_state[c]
            av = ps_av.tile([128, 512], f32, name="av", tag="av")
            sm = ps_sum.tile([128, 512], f32, name="sum", tag="sum")

            def s_exp(j):
                jd = j - 4 * c
                vco = jd * 128 if jd > 0 else 0
                kt = kt_t[hp][j // 4]
                kc = (j % 4) * 128
                ss = ps_s.tile([128, 1024], f32, name="s", tag="s")
                nc.tensor.matmul(ss[:, vco:512],
                                 kt[0:64, kc:kc + 128],
                                 qt_c[hp][0:64, vco:512],
                                 start=True, stop=True)
                nc.tensor.matmul(ss[:, 512 + vco:1024],
                                 kt[64:128, kc:kc + 128],
                                 qt_c[hp][64:128, vco:512],
                                 start=True, stop=True)
                et = exp_pool.tile([128, 1024], bf16, name="e", tag="e")
                if vco:
                    in3 = ss[:].rearrange("p (b q) -> p b q", b=2)[:, :, vco:512]
                    out3 = et[:].rearrange("p (b q) -> p b q", b=2)[:, :, vco:512]
                    nc.scalar.activation(out3, in3, Exp, scale=SCALE)
                else:
                    nc.scalar.activation(et[:], ss[:], Exp, scale=SCALE)
                return et

            def emit_avsm(j, et):
                jd = j - 4 * c
                vco = jd * 128 if jd > 0 else 0
                first, last = (j == 0), (j == njt - 1)
                nc.tensor.matmul(av[0:64, vco:512],
                                 v_t[j][:, hp * 128:hp * 128 + 64],
                                 et[:, vco:512],
                                 start=first, stop=last)
                nc.tensor.matmul(av[64:128, vco:512],
                                 v_t[j][:, hp * 128 + 64:hp * 128 + 128],
                                 et[:, 512 + vco:1024],
                                 start=first, stop=last)
                nc.tensor.matmul(sm[0:64, vco:512],
                                 ones_t[:, 0:64],
                                 et[:, vco:512],
                                 start=first, stop=last)
                nc.tensor.matmul(sm[64:128, vco:512],
                                 ones_t[:, 0:64],
                                 et[:, 512 + vco:1024],
                                 start=first, stop=last)

            prev = None
            for m in range(njt // 2):
                et0 = s_exp(2 * m)
                et1 = s_exp(2 * m + 1)
                for _ in range(stride):
                    if fillers:
                        fillers.popleft()()
                if prev is not None:
                    emit_avsm(2 * m - 2, prev[0])
                    emit_avsm(2 * m - 1, prev[1])
                prev = (et0, et1)
            for _ in range(stride):
                if fillers:
                    fillers.popleft()()
            emit_avsm(njt - 2, prev[0])
            emit_avsm(njt - 1, prev[1])
            rc = rc_pool.tile([128, 512], f32, name="recip", tag="recip")
            nc.vector.reciprocal_approx_fast(rc[:], sm[:])
            at = at_pool.tile([128, 512], bf16, name=f"at{hp}", tag=f"at{hp}")
            nc.vector.tensor_mul(at[:], av[:], rc[:])
            at_state[c].append(at)

        # Phase plan: attention units blended with fine-grained qkv/proj
        # filler units so the PE stays dense.
        phases = [
            ([(0, 0), (0, 1), (0, 2), (0, 3)], qkv_units(1)),
            ([(1, 0), (1, 1), (1, 2), (1, 3)], qkv_units(2)),
            ([(2, 0), (2, 1), (2, 2), (2, 3)], qkv_units(3)),
            ([(3, 0), (3, 1), (3, 2), (3, 3)],
             proj_units(0) + proj_units(1) + proj_units(2)),
        ]
        for units, filler_list in phases:
            fillers = deque(filler_list)
            # one filler burst per key-tile pair (+1 per unit end)
            bursts = sum(4 * c + 4 for c, hp in units) // 2 + len(units)
            stride = max(1, -(-len(filler_list) // bursts))
            for (c, hp) in units:
                attn_unit(c, hp, fillers, stride)
            while fillers:
                fillers.popleft()()

        # ---- final chunk's projection (nothing left to hide it behind) ----
        for u in proj_units(NCH - 1, sync_only=True):
            u()

    nc.compile()
    return nc


def _get_nc():
    if "nc" not in _CACHE:
        _CACHE["nc"] = _build()
    return _CACHE["nc"]


def _in_maps(x, w_qkv, w_proj):
    bf = ml_dtypes.bfloat16
    wr = w_qkv.reshape(C, 3, H, HD)
    wpr = w_proj.reshape(H, HD, C)
    maps = []
    for core in range(NCORES):
        b, g = core // 2, core % 2
        hs = slice(g * HPC, (g + 1) * HPC)
        maps.append({
            "xT": np.ascontiguousarray(x[b].T).astype(bf),
            "wq": np.ascontiguousarray(wr[:, 0, hs, :].reshape(C, F)).astype(bf),
            "wk": np.ascontiguousarray(wr[:, 1, hs, :].reshape(C, F)).astype(bf),
            "wv": np.ascontiguousarray(wr[:, 2, hs, :].reshape(C, F)).astype(bf),
            "wp": np.ascontiguousarray(wpr[hs].reshape(F, C)).astype(bf),
            "ones": np.ones((128, 64), dtype=bf),
        })
    return maps


def kernel(x, w_qkv, w_proj, b_proj, _trace=False):
    from concourse.bass_utils import run_bass_kernel_spmd

    x = np.asarray(x, dtype=np.float32)
    w_qkv = np.asarray(w_qkv, dtype=np.float32)
    w_proj = np.asarray(w_proj, dtype=np.float32)
    b_proj = np.asarray(b_proj, dtype=np.float32)

    nc = _get_nc()
    in_maps = _in_maps(x, w_qkv, w_proj)
    try:
        res = run_bass_kernel_spmd(nc, in_maps, list(range(NCORES)),
                                   trace=_trace)
    except Exception:
        # Device may be wedged from a prior run; reset the axon-side NRT
        # and retry once.
        try:
            import ctypes
            import jax
            lib = ctypes.CDLL("/opt/axon/libaxon_pjrt.so")
            jax.devices()
            lib.axon_reset.restype = ctypes.c_int64
            lib.axon_reset()
        except Exception:
            pass
        res = run_bass_kernel_spmd(nc, in_maps, list(range(NCORES)),
                                   trace=_trace)
    out = np.empty((B, N, C), dtype=np.float32)
    for b in range(B):
        out[b] = res.results[2 * b]["out"] + res.results[2 * b + 1]["out"]
    out += b_proj.reshape(1, 1, C)
    if _trace:
        return out, res
    return out
